# revision 19
# baseline (speedup 1.0000x reference)
"""Trainium2 Bass kernel for nn_Decoder (worker/task label-probability decoder).

Math:
    worker_feature = inputs[:2048, :64]          # [Wn, A]
    tau            = inputs[2048:, :16]          # [T, L]
    p1 = sigmoid(worker_feature @ W + b)         # [Wn, 1]
    p2 = (1 - p1) / (L - 1)
    P[i, j, l] = p1[i]^tau[j,l] * p2[i]^(1 - tau[j,l])
               = exp(a[i] * tau[j,l] + c[i]),  a = ln p1 - ln p2, c = ln p2

Sharding: pure data parallel over the worker axis (dim 0), 256 workers per
core across 8 cores; tau/W/b replicated. No communication.

Per-core schedule: workers live on SBUF partitions (2 groups of 128), the
flattened task axis streams through PSUM in 2048-column tiles. tau arrives
as a [16, 2048] stripe tile via small column-sliced loads spread over the
DMA queues; the tensor engine replicates each stripe to all 128 partitions
with an exact float32r selector matmul (e_s^T @ tau -> PSUM). The
Exp(a*tau + c) activations run with per-partition scale/bias on the SP/PE
sequencer queues, and results stream to HBM as chunk-contiguous writes
(output laid out [G, H, 128, F/H] so each DMA's DRAM footprint is one
contiguous block; the host-side gather undoes the chunking). After tile
scheduling freezes the sync graph, contiguous DRAM-side write APs are
refactored into a fine-grained descriptor form on the DMA fast path.
"""

import numpy as np

try:
    import concourse.bass as bass  # noqa: F401
except ImportError:  # fall back to the container's repo checkout
    import sys

    for _p in ("/root/.axon_site/_ro/trn_rl_repo", "/opt/trn_rl_repo"):
        if _p not in sys.path:
            sys.path.append(_p)

import concourse.bass as bass
import concourse.tile as tile
from concourse import mybir
from concourse.bass_utils import run_bass_kernel_spmd

WN = 2048  # workers total
TN = 2048  # tasks
L = 16  # edge types / labels
A = 64  # ability features
NCORES = 8
WPC = WN // NCORES  # workers per core (256)
G = WPC // 128  # partition groups per core (2)
F = TN * L  # flattened task axis (32768)

AA = A + 1  # features + bias column folded in
NST = 16  # tau stripes on SBUF partitions
STW = F // NST  # stripe width (2048)
H = 4  # output chunks (out tensor [G, H, 128, F/H])
CHW = F // H  # chunk width (8192)
MM = 512  # matmul moving-operand columns per instruction
PSW = 2048  # psum tile width (4 banks)

_AF = mybir.ActivationFunctionType
_f32 = mybir.dt.float32
_f32r = mybir.dt.float32r

MM_ENGINES = ("tensor", "vector", "gpsimd", "scalar")  # replication queues
WRITE_ENGINES = ("sync", "tensor", "vector", "gpsimd", "scalar")


class _TC(tile.TileContext):
    """TileContext with two post-scheduling passes.

    1. `_refactor_write_aps`: every output-write DMA covers one contiguous
       DRAM block and walks it in ascending address order; refactor that
       walk into a [[128, n], [1, 128]] descriptor form (identical address
       sequence, element for element, so the frozen sync graph and the
       SBUF-side pattern are untouched).
    2. `_split_multi_waits`: the walrus build in this container rejects any
       instruction carrying more than one sync-wait command; rewrite every
       multi-wait instruction into a chain of same-engine NOPs (one wait
       each) followed by the instruction with the final wait.
    """

    def _drain_and_barrier(self, tick_clock, wait_clock):
        super()._drain_and_barrier(tick_clock, wait_clock)
        self._refactor_write_aps()
        self._split_multi_waits()

    # -- pass 1: fine-grained descriptor form for contiguous DRAM writes ---

    @staticmethod
    def _contig_ascending(ap):
        """Return total elements if `ap` walks one contiguous DRAM block in
        ascending address order (strictly nested row-major dims)."""
        total = 1
        for stride, num in reversed([list(d) for d in ap]):
            if stride != total:
                return None
            total *= num
        return total

    def _refactor_write_aps(self):
        for fn in self.nc.m.functions:
            for bb in fn.blocks:
                for inst in bb.instructions:
                    if not isinstance(inst, mybir.InstDMACopy):
                        continue
                    o = inst.outs[0]
                    if o.memref != "out":
                        continue
                    total = self._contig_ascending(o.ap)
                    if total is None or total % 128 != 0 or total < 1024:
                        continue
                    new_ap = [[128, total // 128], [1, 128]]
                    o.ap = new_ap
                    if o.bass_ap is not None:
                        o.bass_ap.ap = mybir.VecI64Pair(new_ap)

    # -- pass 2: walrus single-wait legalization ---------------------------

    def _fresh_nop(self, engine):
        inst = self.nc.engines[engine].nop(nofuse=True).ins
        self.nc.cur_bb.bb.instructions.remove(inst)
        return inst

    def _split_multi_waits(self):
        for fn in self.nc.m.functions:
            for bb in fn.blocks:
                snapshot = list(bb.instructions)
                if not any(
                    inst.sync_info and len(inst.sync_info.on_wait) > 1
                    for inst in snapshot
                ):
                    continue
                new = []
                for inst in snapshot:
                    si = inst.sync_info
                    if si is not None and si.on_wait and len(si.on_wait) > 1:
                        waits = list(si.on_wait)
                        si.on_wait = waits[-1:]
                        inst.sync_info = si
                        for wt in waits[:-1]:
                            nop = self._fresh_nop(inst.engine)
                            nop.sync_info = mybir.SyncInfo(on_wait=[wt], on_update=[])
                            new.append(nop)
                    new.append(inst)
                bb.instructions[:] = new


def _act(nc, eng, out_ap, in_ap, func, bias=0.0, scale=1.0):
    """Emit an InstActivation on an arbitrary engine queue."""
    ins = [eng.lower_ap(in_ap)]
    for v in (bias, scale, 0.0):
        if isinstance(v, bass.AP):
            ins.append(eng.lower_ap(v))
        else:
            ins.append(mybir.ImmediateValue(dtype=_f32, value=float(v)))
    inst = mybir.InstActivation(
        name=nc.get_next_instruction_name(),
        func=func,
        ins=ins,
        outs=[eng.lower_ap(out_ap)],
    )
    return eng.add_instruction(inst)


def _mm(nc, eng, out_ap, lhsT, rhs):
    """Emit an InstMatmult on an arbitrary engine queue (stationary lhsT,
    moving rhs), mirroring nc.tensor.matmul's lowering."""
    ifmap_ap = eng.lower_ap(rhs.opt({0}), opt=False)
    weights_ap = eng.lower_ap(lhsT.opt({0}), opt=False, for_matmul_weights=True)
    inst = mybir.InstMatmult(
        name=nc.get_next_instruction_name(),
        replication_resolution=0,
        replication_shift_amnt=0,
        replication_num_rows=0,
        start_tensor_calc=True,
        stop_tensor_calc=True,
        ins=[ifmap_ap, weights_ap],
        outs=[eng.lower_ap(out_ap)],
        perf_mode=None,
        is_transpose=False,
        ifmap_quant_offset=None,
        weights_quant_offset=None,
        bass_skip_group_check=False,
        tile_position=(0, 0),
        tile_size=(32, 128),
    )
    return eng.add_instruction(inst)


def build_nc():
    nc = bass.Bass("TRN2")
    # Let DVE issue HWDGE dma_starts as well (codegen policy, not silicon -
    # any engine's NX can trigger the HWDGE RTL).
    nc.hwdge_engines = [
        mybir.EngineType.SP,
        mybir.EngineType.Activation,
        mybir.EngineType.DVE,
        mybir.EngineType.PE,
    ]

    wf = nc.dram_tensor("wf", [WPC, AA], _f32, kind="ExternalInput")
    tau_in = nc.dram_tensor("tau", [F], _f32, kind="ExternalInput")
    w_in = nc.dram_tensor("W", [AA], _f32, kind="ExternalInput")
    sel_in = nc.dram_tensor("sel", [NST, NST * 128], _f32, kind="ExternalInput")
    out = nc.dram_tensor("out", [G, H, 128, CHW], _f32, kind="ExternalOutput")

    with _TC(nc) as tc:
        with (
            tc.tile_pool(name="const", bufs=1) as const,
            tc.tile_pool(name="outs", bufs=2) as outs,
            tc.tile_pool(name="psum", bufs=2, space="PSUM") as psum,
        ):
            # ---- activation-table priming (Exp/Ln share one func set);
            # first thing on the ACT queue so every later activation can be
            # served without a table reload ----
            zeros = const.tile([128, 1], _f32)
            nc.vector.memset(zeros, 0.0)
            prime = const.tile([128, 1], _f32)
            nc.scalar.activation(prime, zeros, _AF.Exp)

            # ---- leading constant loads: worker features (bias folded in as
            # feature column A) on SP, augmented weights broadcast on Pool ----
            wf_sb = const.tile([128, G, AA], _f32)
            nc.sync.dma_start(
                out=wf_sb, in_=wf[:].rearrange("(g p) a -> p g a", p=128)
            )
            w_ap = w_in[:]
            w_sb = const.tile([128, AA], _f32)
            nc.gpsimd.dma_start(
                out=w_sb,
                in_=bass.AP(tensor=w_ap.tensor, offset=w_ap.offset, ap=[[0, 128], [1, AA]]),
            )

            # ---- stripe-selector weights sel[:, s*128:(s+1)*128] = e_s and
            # tau stripes [16, 2048]. Queue e loads its own tau column slice
            # (same-queue consumers need no semaphore round trip) plus one
            # sel slice; sel slice 0 goes out first on the tensor queue since
            # every queue's first matmuls use stripe blocks 0-3. ----
            sel = const.tile([NST, NST * 128], _f32)
            tau_sb = const.tile([NST, STW], _f32)
            for e in range(len(MM_ENGINES)):
                eng = getattr(nc, MM_ENGINES[e])
                if e == 0:
                    eng.dma_start(out=sel[:, : 4 * 128], in_=sel_in[:, : 4 * 128])
                eng.dma_start(
                    out=tau_sb[:, e * MM : (e + 1) * MM],
                    in_=bass.AP(
                        tensor=tau_in[:].tensor,
                        offset=e * MM,
                        ap=[[STW, NST], [1, MM]],
                    ),
                )
                if e > 0:
                    eng.dma_start(
                        out=sel[:, e * 4 * 128 : (e + 1) * 4 * 128],
                        in_=sel_in[:, e * 4 * 128 : (e + 1) * 4 * 128],
                    )

            # ---- per-worker scalars: a = ln p1 - ln p2, c = ln p2.
            # p2 follows the reference's fp32 dataflow ((1 - p1)/15) so the
            # cancellation error for saturated workers matches bit-for-bit. --
            x = const.tile([128, G], _f32)
            scr = const.tile([128, G, AA], _f32)
            for g in range(G):
                nc.vector.tensor_tensor_reduce(
                    scr[:, g, :],
                    wf_sb[:, g, :],
                    w_sb,
                    1.0,
                    0.0,
                    mybir.AluOpType.mult,
                    mybir.AluOpType.add,
                    x[:, g : g + 1],
                )
            # e = exp(-x); p1 = 1/(1+e); p2 = (1-p1)/15; pack [p1|p2] for Ln.
            # The whole chain rides the SP queue (same-queue deps need no
            # semaphore round trips); only the final subtract sits on DVE.
            e = const.tile([128, G], _f32)
            _act(nc, nc.sync, e, x, _AF.Exp, bias=0.0, scale=-1.0)
            e1 = const.tile([128, G], _f32)
            _act(nc, nc.sync, e1, e, _AF.Copy, bias=1.0, scale=1.0)
            pack = const.tile([128, 2 * G], _f32)
            bass.BassVectorEngine.reciprocal(nc.sync, pack[:, 0:G], e1)
            _act(
                nc,
                nc.sync,
                pack[:, G : 2 * G],
                pack[:, 0:G],
                _AF.Copy,
                bias=1.0 / (L - 1),
                scale=-1.0 / (L - 1),
            )
            lp = const.tile([128, 2 * G], _f32)
            _act(nc, nc.sync, lp, pack, _AF.Ln)
            lp2 = lp[:, G : 2 * G]
            a_sb = const.tile([128, G], _f32)
            nc.vector.tensor_sub(a_sb, lp[:, 0:G], lp2)

            # ---- main loop: selector-matmul bcast -> Exp -> stream out.
            # Column slice 512*(4j+e) is replicated by queue e (its own pair
            # of PSUM banks): lhsT = sel block j, rhs = tau cols
            # [e*512, (e+1)*512) of stripe j. Exp activations ride the SP
            # queue; each chunk's two group-writes go out round-robin. ----
            NE = len(MM_ENGINES)
            wr = 0
            for h in range(H):
                ots = [
                    outs.tile([128, CHW], _f32, tag=f"ot{g}", name=f"ot{g}_{h}")
                    for g in range(G)
                ]
                for j in range(4 * h, 4 * h + 4):  # stripes of this chunk
                    for pair in range(2):  # unit pairs (e in {0,1} / {2,3})
                        col = (4 * j + 2 * pair) * MM
                        pt = psum.tile(
                            [128, 2 * MM], _f32, tag=f"pp{pair}", name=f"pt{j}_{pair}"
                        )
                        for half in range(2):
                            e = 2 * pair + half
                            _mm(
                                nc,
                                getattr(nc, MM_ENGINES[e]),
                                pt[:, half * MM : (half + 1) * MM],
                                sel[:, j * 128 : (j + 1) * 128].bitcast(_f32r),
                                tau_sb[:, e * MM : (e + 1) * MM].bitcast(_f32r),
                            )
                        for g in range(G):
                            _act(
                                nc,
                                nc.sync,
                                ots[g][:, col - h * CHW : col - h * CHW + 2 * MM],
                                pt,
                                _AF.Exp,
                                bias=lp2[:, g : g + 1],
                                scale=a_sb[:, g : g + 1],
                            )
                for g in range(G):
                    getattr(nc, WRITE_ENGINES[wr % len(WRITE_ENGINES)]).dma_start(
                        out=out[g, h], in_=ots[g]
                    )
                    wr += 1
    return nc


def _selector():
    """sel[k, s*128 + p] = 1 if k == s else 0  (stripe-selector weights)."""
    sel = np.zeros((NST, NST * 128), dtype=np.float32)
    for s in range(NST):
        sel[s, s * 128 : (s + 1) * 128] = 1.0
    return sel


_NC = None


def kernel(inputs, W, b, worker_num=WN, task_num=TN, edge_type=L, ability_num=A, **_kw):
    global _NC
    inputs = np.ascontiguousarray(np.asarray(inputs, dtype=np.float32))
    W = np.asarray(W, dtype=np.float32).reshape(A)
    b = np.asarray(b, dtype=np.float32).reshape(1)
    assert inputs.shape == (WN + TN, A)

    wf = np.concatenate(
        [inputs[:WN, :A], np.ones((WN, 1), dtype=np.float32)], axis=1
    )
    W_aug = np.concatenate([W, b]).astype(np.float32)
    tau = np.ascontiguousarray(inputs[WN:, :L].reshape(F))
    sel = _selector()

    if _NC is None:
        _NC = build_nc()

    in_maps = [
        {
            "wf": np.ascontiguousarray(wf[k * WPC : (k + 1) * WPC]),
            "tau": tau,
            "W": W_aug,
            "sel": sel,
        }
        for k in range(NCORES)
    ]
    res = run_bass_kernel_spmd(_NC, in_maps, core_ids=list(range(NCORES)))
    parts = []
    for r in res.results:
        o = r["out"]  # [G, H, 128, CHW]
        o = o.transpose(0, 2, 1, 3).reshape(WPC, TN, L)
        parts.append(o)
    return np.concatenate(parts, axis=0)


# revision 20
# speedup vs baseline: 1.2161x; 1.2161x over previous
"""Trainium2 Bass kernel for nn_Decoder (worker/task label-probability decoder).

Math:
    worker_feature = inputs[:2048, :64]          # [Wn, A]
    tau            = inputs[2048:, :16]          # [T, L]
    p1 = sigmoid(worker_feature @ W + b)         # [Wn, 1]
    p2 = (1 - p1) / (L - 1)
    P[i, j, l] = p1[i]^tau[j,l] * p2[i]^(1 - tau[j,l])
               = exp(a[i] * tau[j,l] + c[i]),  a = ln p1 - ln p2, c = ln p2

Sharding: pure data parallel over the worker axis (dim 0), 256 workers per
core across 8 cores; tau/W/b replicated. No communication.

Per-core schedule: workers live on SBUF partitions (2 groups of 128), the
flattened task axis streams through PSUM in 2048-column tiles. tau arrives
as a [16, 2048] stripe tile via small column-sliced loads spread over the
DMA queues; the tensor engine replicates each stripe to all 128 partitions
with an exact float32r selector matmul (e_s^T @ tau -> PSUM). The
Exp(a*tau + c) activations run with per-partition scale/bias on the SP/PE
sequencer queues, and results stream to HBM as chunk-contiguous writes
(output laid out [G, H, 128, F/H] so each DMA's DRAM footprint is one
contiguous block; the host-side gather undoes the chunking). After tile
scheduling freezes the sync graph, contiguous DRAM-side write APs are
refactored into a fine-grained descriptor form on the DMA fast path.
"""

import numpy as np

try:
    import concourse.bass as bass  # noqa: F401
except ImportError:  # fall back to the container's repo checkout
    import sys

    for _p in ("/root/.axon_site/_ro/trn_rl_repo", "/opt/trn_rl_repo"):
        if _p not in sys.path:
            sys.path.append(_p)

import concourse.bass as bass
import concourse.tile as tile
from concourse import mybir
from concourse.bass_utils import run_bass_kernel_spmd

WN = 2048  # workers total
TN = 2048  # tasks
L = 16  # edge types / labels
A = 64  # ability features
NCORES = 8
WPC = WN // NCORES  # workers per core (256)
G = WPC // 128  # partition groups per core (2)
F = TN * L  # flattened task axis (32768)

AA = A + 1  # features + bias column folded in
NST = 16  # tau stripes on SBUF partitions
STW = F // NST  # stripe width (2048)
H = 4  # output chunks (out tensor [G, H, 128, F/H])
CHW = F // H  # chunk width (8192)
MM = 512  # matmul moving-operand columns per instruction
PSW = 2048  # psum tile width (4 banks)

_AF = mybir.ActivationFunctionType
_f32 = mybir.dt.float32
_f32r = mybir.dt.float32r

MM_ENGINES = ("tensor", "vector", "gpsimd", "scalar")  # replication queues
WRITE_ENGINES = ("sync", "tensor", "vector", "gpsimd", "scalar")


class _TC(tile.TileContext):
    """TileContext with two post-scheduling passes.

    1. `_refactor_write_aps`: every output-write DMA covers one contiguous
       DRAM block and walks it in ascending address order; refactor that
       walk into a [[128, n], [1, 128]] descriptor form (identical address
       sequence, element for element, so the frozen sync graph and the
       SBUF-side pattern are untouched).
    2. `_split_multi_waits`: the walrus build in this container rejects any
       instruction carrying more than one sync-wait command; rewrite every
       multi-wait instruction into a chain of same-engine NOPs (one wait
       each) followed by the instruction with the final wait.
    """

    def _drain_and_barrier(self, tick_clock, wait_clock):
        super()._drain_and_barrier(tick_clock, wait_clock)
        self._refactor_write_aps()
        self._split_multi_waits()

    # -- pass 1: fine-grained descriptor form for contiguous DRAM writes ---

    @staticmethod
    def _contig_ascending(ap):
        """Return total elements if `ap` walks one contiguous DRAM block in
        ascending address order (strictly nested row-major dims)."""
        total = 1
        for stride, num in reversed([list(d) for d in ap]):
            if stride != total:
                return None
            total *= num
        return total

    def _refactor_write_aps(self):
        for fn in self.nc.m.functions:
            for bb in fn.blocks:
                for inst in bb.instructions:
                    if not isinstance(inst, mybir.InstDMACopy):
                        continue
                    o = inst.outs[0]
                    if o.memref != "out":
                        continue
                    total = self._contig_ascending(o.ap)
                    if total is None or total % 128 != 0 or total < 1024:
                        continue
                    new_ap = [[128, total // 128], [1, 128]]
                    o.ap = new_ap
                    if o.bass_ap is not None:
                        o.bass_ap.ap = mybir.VecI64Pair(new_ap)

    # -- pass 2: walrus single-wait legalization ---------------------------

    def _fresh_nop(self, engine):
        inst = self.nc.engines[engine].nop(nofuse=True).ins
        self.nc.cur_bb.bb.instructions.remove(inst)
        return inst

    def _split_multi_waits(self):
        for fn in self.nc.m.functions:
            for bb in fn.blocks:
                snapshot = list(bb.instructions)
                if not any(
                    inst.sync_info and len(inst.sync_info.on_wait) > 1
                    for inst in snapshot
                ):
                    continue
                new = []
                for inst in snapshot:
                    si = inst.sync_info
                    if si is not None and si.on_wait and len(si.on_wait) > 1:
                        waits = list(si.on_wait)
                        si.on_wait = waits[-1:]
                        inst.sync_info = si
                        for wt in waits[:-1]:
                            nop = self._fresh_nop(inst.engine)
                            nop.sync_info = mybir.SyncInfo(on_wait=[wt], on_update=[])
                            new.append(nop)
                    new.append(inst)
                bb.instructions[:] = new


def _act(nc, eng, out_ap, in_ap, func, bias=0.0, scale=1.0):
    """Emit an InstActivation on an arbitrary engine queue."""
    ins = [eng.lower_ap(in_ap)]
    for v in (bias, scale, 0.0):
        if isinstance(v, bass.AP):
            ins.append(eng.lower_ap(v))
        else:
            ins.append(mybir.ImmediateValue(dtype=_f32, value=float(v)))
    inst = mybir.InstActivation(
        name=nc.get_next_instruction_name(),
        func=func,
        ins=ins,
        outs=[eng.lower_ap(out_ap)],
    )
    return eng.add_instruction(inst)


def _mm(nc, eng, out_ap, lhsT, rhs):
    """Emit an InstMatmult on an arbitrary engine queue (stationary lhsT,
    moving rhs), mirroring nc.tensor.matmul's lowering."""
    ifmap_ap = eng.lower_ap(rhs.opt({0}), opt=False)
    weights_ap = eng.lower_ap(lhsT.opt({0}), opt=False, for_matmul_weights=True)
    inst = mybir.InstMatmult(
        name=nc.get_next_instruction_name(),
        replication_resolution=0,
        replication_shift_amnt=0,
        replication_num_rows=0,
        start_tensor_calc=True,
        stop_tensor_calc=True,
        ins=[ifmap_ap, weights_ap],
        outs=[eng.lower_ap(out_ap)],
        perf_mode=None,
        is_transpose=False,
        ifmap_quant_offset=None,
        weights_quant_offset=None,
        bass_skip_group_check=False,
        tile_position=(0, 0),
        tile_size=(32, 128),
    )
    return eng.add_instruction(inst)


def build_nc():
    nc = bass.Bass("TRN2")
    # Let DVE issue HWDGE dma_starts as well (codegen policy, not silicon -
    # any engine's NX can trigger the HWDGE RTL).
    nc.hwdge_engines = [
        mybir.EngineType.SP,
        mybir.EngineType.Activation,
        mybir.EngineType.DVE,
        mybir.EngineType.PE,
    ]

    wf = nc.dram_tensor("wf", [WPC, AA], _f32, kind="ExternalInput")
    tau_in = nc.dram_tensor("tau", [F], _f32, kind="ExternalInput")
    w_in = nc.dram_tensor("W", [AA], _f32, kind="ExternalInput")
    sel_in = nc.dram_tensor("sel", [NST, NST * 128], _f32, kind="ExternalInput")
    out = nc.dram_tensor("out", [G, H, 128, CHW], _f32, kind="ExternalOutput")

    with _TC(nc) as tc:
        with (
            tc.tile_pool(name="const", bufs=1) as const,
            tc.tile_pool(name="outs", bufs=2) as outs,
            tc.tile_pool(name="psum", bufs=2, space="PSUM") as psum,
        ):
            # ---- activation-table priming (Exp/Ln share one func set);
            # first thing on the ACT queue so every later activation can be
            # served without a table reload ----
            zeros = const.tile([128, 1], _f32)
            nc.vector.memset(zeros, 0.0)
            prime = const.tile([128, 1], _f32)
            nc.scalar.activation(prime, zeros, _AF.Exp)

            # ---- leading constant loads: worker features (bias folded in as
            # feature column A) on SP, augmented weights broadcast on Pool ----
            wf_sb = const.tile([128, G, AA], _f32)
            nc.sync.dma_start(
                out=wf_sb, in_=wf[:].rearrange("(g p) a -> p g a", p=128)
            )
            w_ap = w_in[:]
            w_sb = const.tile([128, AA], _f32)
            nc.gpsimd.dma_start(
                out=w_sb,
                in_=bass.AP(tensor=w_ap.tensor, offset=w_ap.offset, ap=[[0, 128], [1, AA]]),
            )

            # ---- stripe-selector weights sel[:, s*128:(s+1)*128] = e_s and
            # tau stripes [16, 2048]. Queue e loads its own tau column slice
            # (same-queue consumers need no semaphore round trip) plus one
            # sel slice; sel slice 0 goes out first on the tensor queue since
            # every queue's first matmuls use stripe blocks 0-3. ----
            sel = const.tile([NST, NST * 128], _f32)
            tau_sb = const.tile([NST, STW], _f32)
            for e in range(len(MM_ENGINES)):
                eng = getattr(nc, MM_ENGINES[e])
                if e == 0:
                    eng.dma_start(out=sel[:, : 4 * 128], in_=sel_in[:, : 4 * 128])
                eng.dma_start(
                    out=tau_sb[:, e * MM : (e + 1) * MM],
                    in_=bass.AP(
                        tensor=tau_in[:].tensor,
                        offset=e * MM,
                        ap=[[STW, NST], [1, MM]],
                    ),
                )
                if e > 0:
                    eng.dma_start(
                        out=sel[:, e * 4 * 128 : (e + 1) * 4 * 128],
                        in_=sel_in[:, e * 4 * 128 : (e + 1) * 4 * 128],
                    )

            # ---- per-worker scalars: a = ln p1 - ln p2, c = ln p2.
            # p2 follows the reference's fp32 dataflow ((1 - p1)/15) so the
            # cancellation error for saturated workers matches bit-for-bit. --
            x = const.tile([128, G], _f32)
            scr = const.tile([128, G, AA], _f32)
            for g in range(G):
                nc.vector.tensor_tensor_reduce(
                    scr[:, g, :],
                    wf_sb[:, g, :],
                    w_sb,
                    1.0,
                    0.0,
                    mybir.AluOpType.mult,
                    mybir.AluOpType.add,
                    x[:, g : g + 1],
                )
            # e = exp(-x); p1 = 1/(1+e); p2 = (1-p1)/15; pack [p1|p2] for Ln.
            # The whole chain rides the SP queue (same-queue deps need no
            # semaphore round trips); only the final subtract sits on DVE.
            e = const.tile([128, G], _f32)
            _act(nc, nc.sync, e, x, _AF.Exp, bias=0.0, scale=-1.0)
            e1 = const.tile([128, G], _f32)
            _act(nc, nc.sync, e1, e, _AF.Copy, bias=1.0, scale=1.0)
            pack = const.tile([128, 2 * G], _f32)
            bass.BassVectorEngine.reciprocal(nc.sync, pack[:, 0:G], e1)
            _act(
                nc,
                nc.sync,
                pack[:, G : 2 * G],
                pack[:, 0:G],
                _AF.Copy,
                bias=1.0 / (L - 1),
                scale=-1.0 / (L - 1),
            )
            lp = const.tile([128, 2 * G], _f32)
            _act(nc, nc.sync, lp, pack, _AF.Ln)
            lp2 = lp[:, G : 2 * G]
            a_sb = const.tile([128, G], _f32)
            nc.vector.tensor_sub(a_sb, lp[:, 0:G], lp2)

            # ---- main loop: selector-matmul bcast -> Exp -> stream out.
            # Column slice 512*(4j+e) is replicated by queue e (its own pair
            # of PSUM banks): lhsT = sel block j, rhs = tau cols
            # [e*512, (e+1)*512) of stripe j. Exp activations ride the SP
            # queue; each chunk's two group-writes go out round-robin. ----
            NE = len(MM_ENGINES)
            wr = 0
            for h in range(H):
                ots = [
                    outs.tile([128, CHW], _f32, tag=f"ot{g}", name=f"ot{g}_{h}")
                    for g in range(G)
                ]
                for j in range(4 * h, 4 * h + 4):  # stripes of this chunk
                    for e in range(NE):
                        col = (4 * j + e) * MM  # absolute output column
                        eng = getattr(nc, MM_ENGINES[e])
                        pt = psum.tile(
                            [128, MM], _f32, tag=f"pe{e}", name=f"pt{j}_{e}"
                        )
                        _mm(
                            nc,
                            eng,
                            pt[:],
                            sel[:, j * 128 : (j + 1) * 128].bitcast(_f32r),
                            tau_sb[:, e * MM : (e + 1) * MM].bitcast(_f32r),
                        )
                        for g in range(G):
                            _act(
                                nc,
                                nc.sync,
                                ots[g][:, col - h * CHW : col - h * CHW + MM],
                                pt,
                                _AF.Exp,
                                bias=lp2[:, g : g + 1],
                                scale=a_sb[:, g : g + 1],
                            )
                for g in range(G):
                    getattr(nc, WRITE_ENGINES[wr % len(WRITE_ENGINES)]).dma_start(
                        out=out[g, h], in_=ots[g]
                    )
                    wr += 1
    return nc


def _selector():
    """sel[k, s*128 + p] = 1 if k == s else 0  (stripe-selector weights)."""
    sel = np.zeros((NST, NST * 128), dtype=np.float32)
    for s in range(NST):
        sel[s, s * 128 : (s + 1) * 128] = 1.0
    return sel


_NC = None


def kernel(inputs, W, b, worker_num=WN, task_num=TN, edge_type=L, ability_num=A, **_kw):
    global _NC
    inputs = np.ascontiguousarray(np.asarray(inputs, dtype=np.float32))
    W = np.asarray(W, dtype=np.float32).reshape(A)
    b = np.asarray(b, dtype=np.float32).reshape(1)
    assert inputs.shape == (WN + TN, A)

    wf = np.concatenate(
        [inputs[:WN, :A], np.ones((WN, 1), dtype=np.float32)], axis=1
    )
    W_aug = np.concatenate([W, b]).astype(np.float32)
    tau = np.ascontiguousarray(inputs[WN:, :L].reshape(F))
    sel = _selector()

    if _NC is None:
        _NC = build_nc()

    in_maps = [
        {
            "wf": np.ascontiguousarray(wf[k * WPC : (k + 1) * WPC]),
            "tau": tau,
            "W": W_aug,
            "sel": sel,
        }
        for k in range(NCORES)
    ]
    res = run_bass_kernel_spmd(_NC, in_maps, core_ids=list(range(NCORES)))
    parts = []
    for r in res.results:
        o = r["out"]  # [G, H, 128, CHW]
        o = o.transpose(0, 2, 1, 3).reshape(WPC, TN, L)
        parts.append(o)
    return np.concatenate(parts, axis=0)
